# revision 9
# baseline (speedup 1.0000x reference)
"""FFT conv block (rfft2 -> per-channel complex multiply -> irfft2) on 8 trn2
cores — v4: bf16 I/O, TB-once radix-2 butterflies, nyquist folded into the
dead imag kh=0 slot, batched T3 moving matrices.

Math per (b,c) image [256, 256] (sharded over channels, 8 ch/core):
  T1: rfft over H via matmul (bf16 x, bf16 consts), even/odd w columns:
      Y1e[we, kh], Y1o[wo, kh]  (kh 0..128 r|i, f32 psum -> bf16)
  T2: radix-2 fft over W: A = DFT128(even w) psum, TB = twiddled DFT128(odd)
      psum; butterflies lo = A+TB (DVE), hi = A-TB (Pool) -> bf16
  wm: Yw = Y2 * Weff (bf16, DVE, weights broadcast over images)
  S/D: S = Yw_lo + Yw_hi, D = Yw_lo - Yw_hi (radix-2 DIF for the inverse)
  nyquist fold: c0 := c_kh0 + i*c_nyq per column (S and D), so the inverse-W
      transform lands Z_kh0 in the real row 0 and Z_nyq in the imag row 0;
      B4's kh=0 row (identically zero) is replaced by the nyquist row a4n.
  T3: Z[kh, (r|i), m] = S @ [Ce|Se] + Sconj @ [-Se|Ce] style batched matmuls
  T4: y = A4^T Zr + B4^T Zi through an interleaving AP (f32r matmuls),
      evicted to bf16 and DMA'd out.
Verified vs reference: ~5.3e-3 rel rms (bf16-dominated), tolerance 2e-2.
"""
import sys
sys.path.insert(0, "/opt/trn_rl_repo")
import numpy as np
import ml_dtypes

B, C, H, W = 16, 64, 256, 256
KHF = H // 2 + 1          # 129
KP = 130                  # kh padded (even)
N_CORES = 8
NC_LOC = C // N_CORES     # 8 channels per core
NB = B                    # 16 batch images per channel
G = 8                     # supergroup size (images of same channel)


def _consts_v4():
    f32 = np.float32
    bf = ml_dtypes.bfloat16
    h = np.arange(H)[:, None]
    kh = np.arange(KHF)[None, :]
    th = 2 * np.pi * h * kh / H                      # [H, KHF]
    # packed 256-col M1: [cos kh0..128 (129, col128 = nyquist) | -sin kh1..127]
    # (the kh0 and nyquist imag columns are identically zero)
    M1 = np.concatenate([np.cos(th), -np.sin(th[:, 1:128])], axis=1)  # [256, 256]

    kw = np.arange(128)[None, :]
    we = np.arange(128)[:, None]
    cosE = np.cos(2 * np.pi * (2 * we) * kw / W)     # [we, kw]
    sinE = np.sin(2 * np.pi * (2 * we) * kw / W)
    cosO = np.cos(2 * np.pi * (2 * we + 1) * kw / W)
    sinO = np.sin(2 * np.pi * (2 * we + 1) * kw / W)

    kw2 = np.arange(128)[:, None]
    m = np.arange(128)[None, :]
    Ce = np.cos(2 * np.pi * kw2 * (2 * m) / W) / W
    Se = np.sin(2 * np.pi * kw2 * (2 * m) / W) / W
    Co = np.cos(2 * np.pi * kw2 * (2 * m + 1) / W) / W
    So = np.sin(2 * np.pi * kw2 * (2 * m + 1) / W) / W
    # batched T3 moving mats [kw, 2, m]: one matmul yields [zr | zi] halves
    Me1 = np.stack([Ce, Se], axis=1)                 # stationary = S
    Me2 = np.stack([-Se, Ce], axis=1)                # stationary = Sconj part
    Mo1 = np.stack([Co, So], axis=1)
    Mo2 = np.stack([-So, Co], axis=1)

    kh2 = np.arange(KHF)[:, None]
    hp = np.arange(H)[None, :]
    t4 = 2 * np.pi * kh2 * hp / H
    alpha = np.where((kh2 == 0) | (kh2 == H // 2), 1.0, 2.0)
    A4 = alpha * np.cos(t4) / H                      # [129, 256]
    B4 = -alpha * np.sin(t4) / H
    A4m = A4[0:128].astype(f32)
    B4m = B4[0:128].astype(f32)
    B4m[0, :] = A4[128, :]    # nyquist row rides the dead imag kh=0 slot
    return dict(
        M1=M1.astype(f32).astype(bf),
        cosE=cosE.astype(f32).astype(bf), sinE=sinE.astype(f32).astype(bf),
        nsinE=(-sinE).astype(f32).astype(bf),
        cosO=cosO.astype(f32).astype(bf), sinO=sinO.astype(f32).astype(bf),
        nsinO=(-sinO).astype(f32).astype(bf),
        Me1=Me1.astype(f32).astype(bf), Me2=Me2.astype(f32).astype(bf),
        Mo1=Mo1.astype(f32).astype(bf), Mo2=Mo2.astype(f32).astype(bf),
        A4m=A4m, B4m=B4m,
    )


def _w_eff(wr, wi):
    """wr, wi: [256(kh), 129(kw)] reference layout -> W_eff [129(kh), 256(kw)]."""
    w = wr.astype(np.float64) + 1j * wi.astype(np.float64)
    kh = np.arange(KHF)
    khc = (H - kh) % H
    eff = np.empty((KHF, W), dtype=np.complex128)
    eff[:, 0:W // 2 + 1] = w[0:KHF, :]
    for kwv in range(W // 2 + 1, W):
        eff[:, kwv] = np.conj(w[khc, W - kwv])
    for col in (0, W // 2):
        eff[:, col] = 0.5 * (w[kh, col] + np.conj(w[khc, col]))
    return eff                                        # [129(kh), 256(kw)] complex


def build_nc(nc_loc=NC_LOC, nb=NB, g=G, repeat=1):
    import concourse.mybir as mybir
    import concourse.tile as tile
    from concourse import bacc
    from contextlib import ExitStack

    f32, f32r = mybir.dt.float32, mybir.dt.float32r
    bf16 = mybir.dt.bfloat16
    n_img = nb * nc_loc
    npairs = g // 2
    CN = _consts_v4()

    nc = bacc.Bacc("TRN2", target_bir_lowering=False)
    x_d = nc.dram_tensor("x", [n_img, H, W], bf16, kind="ExternalInput")
    w_d = nc.dram_tensor("w", [nc_loc, 3, 2, 128, KP], bf16, kind="ExternalInput")
    y_d = nc.dram_tensor("y", [n_img, H, W], bf16, kind="ExternalOutput")

    dconst = {k: nc.inline_tensor(v, f"c_{k}") for k, v in CN.items()}

    with tile.TileContext(nc) as tc, ExitStack() as es:
        cpool = es.enter_context(tc.tile_pool(name="const", bufs=1))
        wpool = es.enter_context(tc.tile_pool(name="wpool", bufs=1))
        xrp = es.enter_context(tc.tile_pool(name="xr", bufs=2))
        y1p = es.enter_context(tc.tile_pool(name="y1", bufs=3))
        y2p = es.enter_context(tc.tile_pool(name="y2", bufs=2))
        ywp = es.enter_context(tc.tile_pool(name="yw", bufs=2))
        sdp = es.enter_context(tc.tile_pool(name="sd", bufs=2))
        zsb = es.enter_context(tc.tile_pool(name="zsb", bufs=3))
        ysb = es.enter_context(tc.tile_pool(name="ysb", bufs=3))
        ps1 = es.enter_context(tc.tile_pool(name="ps1", bufs=2, space="PSUM"))
        ps2 = es.enter_context(tc.tile_pool(name="ps2", bufs=1, space="PSUM"))
        ps3 = es.enter_context(tc.tile_pool(name="ps3", bufs=2, space="PSUM"))
        ps4 = es.enter_context(tc.tile_pool(name="ps4", bufs=2, space="PSUM"))

        def load_const(name, dt_):
            src = dconst[name]
            arr = CN[name]
            rows = arr.shape[0]
            inner = list(arr.shape[1:])
            src_is_bf = arr.dtype == ml_dtypes.bfloat16
            tiles = []
            nch = (rows + 127) // 128
            for ch in range(nch):
                r0, r1 = ch * 128, min((ch + 1) * 128, rows)
                t = cpool.tile([r1 - r0] + inner, dt_, name=f"c_{name}{ch}")
                out_ap = t[:] if src_is_bf else t[:].bitcast(f32)
                nc.sync.dma_start(out=out_ap, in_=src[r0:r1])
                tiles.append(t)
            return tiles if nch > 1 else tiles[0]

        m1t = load_const("M1", bf16)               # 2 x [128, 256]
        cosEt = load_const("cosE", bf16)           # [128, 128] each
        sinEt = load_const("sinE", bf16)
        nsinEt = load_const("nsinE", bf16)
        cosOt = load_const("cosO", bf16)
        sinOt = load_const("sinO", bf16)
        nsinOt = load_const("nsinO", bf16)
        Me1t = load_const("Me1", bf16)             # [128, 2, 128] each
        Me2t = load_const("Me2", bf16)
        Mo1t = load_const("Mo1", bf16)
        Mo2t = load_const("Mo2", bf16)
        a4mt = load_const("A4m", f32r)             # [128, 256]
        b4mt = load_const("B4m", f32r)

        # weights: per channel [128(kw), 3(kind), 2(kwc), KP] bf16 on ACT queue
        wt = []
        for cl in range(nc_loc):
            t = wpool.tile([128, 3, 2, KP], bf16, name=f"w{cl}")
            nc.scalar.dma_start(
                out=t, in_=w_d[cl].rearrange("k c p f -> p k c f"))
            wt.append(t)

        def emit_AB(cl, sg0):
            """Phase A (per pair T1+T2 butterflies) + B (wmul, S/D, ny-fold)."""
            y2r = y2p.tile([128, 2, g, KP], bf16, name="y2r")
            y2i = y2p.tile([128, 2, g, KP], bf16, name="y2i")
            for pr in range(npairs):
                img0 = cl * nb + sg0 + 2 * pr
                xr = xrp.tile([128, 2, 2, W], bf16, name="xr")
                nc.sync.dma_start(
                    out=xr,
                    in_=x_d[img0:img0 + 2].rearrange(
                        "i (c p) w -> p i c w", c=2))
                # T1: even/odd w columns; Y1 = [128(w/2), 2j, 2par, 256] bf16
                # (col layout: r kh0..128 w/ col128 = nyquist | i kh1..127)
                y1 = y1p.tile([128, 2, 2, 256], bf16, name="y1")
                for j in range(2):
                    t1 = ps1.tile([128, 2, 256], f32, name="t1ps")
                    for par in range(2):
                        nc.tensor.matmul(
                            t1[:, par], xr[:, j, 0, par::2], m1t[0],
                            start=True, stop=False)
                        nc.tensor.matmul(
                            t1[:, par], xr[:, j, 1, par::2], m1t[1],
                            start=False, stop=True)
                    nc.scalar.copy(out=y1[:, j], in_=t1)
                # T2 radix-2 over w: A (even) and TB (odd, twiddles folded)
                # in PSUM; A evicted to SBUF (ACT), butterflies A+/-TB on
                # DVE/Pool (only TB read from PSUM — single-PSUM-input rule).
                sl = slice(2 * pr, 2 * pr + 2)
                y1e_r = y1[:, :, 0, 0:129]
                y1e_i = y1[:, :, 0, 129:256]
                y1o_r = y1[:, :, 1, 0:129]
                y1o_i = y1[:, :, 1, 129:256]
                for half, y2t in ((0, y2r), (1, y2i)):
                    c1 = cosEt if half == 0 else nsinEt   # on r cols (full)
                    c2 = sinEt if half == 0 else cosEt    # on i cols (1:128)
                    o1 = cosOt if half == 0 else nsinOt
                    o2 = sinOt if half == 0 else cosOt
                    tb = ps2.tile([128, 2, KP], f32, name="tbps")
                    av = ps2.tile([128, 2, KP], f32, name="aps")
                    nc.tensor.matmul(tb[:, :, 0:129], o1, y1o_r, start=True, stop=False)
                    nc.tensor.matmul(tb[:, :, 1:128], o2, y1o_i, start=False, stop=True)
                    nc.tensor.matmul(av[:, :, 0:129], c1, y1e_r, start=True, stop=False)
                    nc.tensor.matmul(av[:, :, 1:128], c2, y1e_i, start=False, stop=True)
                    asb = y1p.tile([128, 2, KP], bf16, name="asb")
                    nc.scalar.copy(out=asb[:, :, 0:129], in_=av[:, :, 0:129])
                    nc.vector.tensor_add(
                        y2t[:, 0, sl, 0:129], asb[:, :, 0:129], tb[:, :, 0:129])
                    nc.gpsimd.tensor_sub(
                        y2t[:, 1, sl, 0:129], asb[:, :, 0:129], tb[:, :, 0:129])
            # ---- phase B: wmul (bf16, DVE) + S/D + nyquist fold ----
            wr_b = wt[cl][:, 0, :, :].unsqueeze(2).broadcast_to([128, 2, g, KP])
            wi_b = wt[cl][:, 1, :, :].unsqueeze(2).broadcast_to([128, 2, g, KP])
            nwi_b = wt[cl][:, 2, :, :].unsqueeze(2).broadcast_to([128, 2, g, KP])
            kv = slice(0, 129)     # valid kh cols (129 never written in psum)
            ta = ywp.tile([128, 2, g, KP], bf16, name="ta")
            tb = ywp.tile([128, 2, g, KP], bf16, name="tb")
            ta2 = ywp.tile([128, 2, g, KP], bf16, name="ta2")
            tb2 = ywp.tile([128, 2, g, KP], bf16, name="tb2")
            ywr = ywp.tile([128, 2, g, KP], bf16, name="ywr")
            ywi = ywp.tile([128, 2, g, KP], bf16, name="ywi")
            nc.vector.tensor_mul(ta[:, :, :, kv], y2r[:, :, :, kv], wr_b[:, :, :, kv])
            nc.vector.tensor_mul(tb[:, :, :, kv], y2i[:, :, :, kv], nwi_b[:, :, :, kv])
            nc.vector.tensor_add(ywr[:, :, :, kv], ta[:, :, :, kv], tb[:, :, :, kv])
            nc.vector.tensor_mul(ta2[:, :, :, kv], y2r[:, :, :, kv], wi_b[:, :, :, kv])
            nc.vector.tensor_mul(tb2[:, :, :, kv], y2i[:, :, :, kv], wr_b[:, :, :, kv])
            nc.vector.tensor_add(ywi[:, :, :, kv], ta2[:, :, :, kv], tb2[:, :, :, kv])
            sr = sdp.tile([128, g, KP], bf16, name="sr")
            si = sdp.tile([128, g, KP], bf16, name="si")
            dr = sdp.tile([128, g, KP], bf16, name="dr")
            di = sdp.tile([128, g, KP], bf16, name="di")
            nc.vector.tensor_add(sr[:, :, kv], ywr[:, 0, :, kv], ywr[:, 1, :, kv])
            nc.gpsimd.tensor_add(si[:, :, kv], ywi[:, 0, :, kv], ywi[:, 1, :, kv])
            nc.vector.tensor_sub(dr[:, :, kv], ywr[:, 0, :, kv], ywr[:, 1, :, kv])
            nc.vector.tensor_sub(di[:, :, kv], ywi[:, 0, :, kv], ywi[:, 1, :, kv])
            # nyquist fold: col0 := col0 (+/-) i*col128 (writes col 0 only,
            # reads col 128 only — order-independent)
            nc.gpsimd.tensor_sub(sr[:, :, 0], sr[:, :, 0], si[:, :, 128])
            nc.gpsimd.tensor_add(si[:, :, 0], si[:, :, 0], sr[:, :, 128])
            nc.gpsimd.tensor_sub(dr[:, :, 0], dr[:, :, 0], di[:, :, 128])
            nc.gpsimd.tensor_add(di[:, :, 0], di[:, :, 0], dr[:, :, 128])
            return dict(sr=sr, si=si, dr=dr, di=di, cl=cl, sg0=sg0)

        def emit_C(st):
            """Phase C: per image T3 (batched moving); per pair T4 + store."""
            sr, si, dr, di = st["sr"], st["si"], st["dr"], st["di"]
            cl, sg0 = st["cl"], st["sg0"]
            kmain = slice(0, 128)
            for pr in range(npairs):
                zzp = zsb.tile([128, 2, 2, 2, 128], f32r, name="zzp")
                for j in range(2):
                    gg = 2 * pr + j
                    z2 = ps3.tile([128, 2, 2, 128], f32, name="z2ps")
                    nc.tensor.matmul(z2[:, 0], sr[:, gg, kmain], Me1t, start=True, stop=False)
                    nc.tensor.matmul(z2[:, 0], si[:, gg, kmain], Me2t, start=False, stop=True)
                    nc.tensor.matmul(z2[:, 1], dr[:, gg, kmain], Mo1t, start=True, stop=False)
                    nc.tensor.matmul(z2[:, 1], di[:, gg, kmain], Mo2t, start=False, stop=True)
                    nc.scalar.copy(out=zzp[:, j], in_=z2)
                # T4: pair-batched; moving reads Z interleaved (m,eo swap)
                yt = ysb.tile([128, 2, 2, W], bf16, name="yt")
                zr_v = zzp[:, :, :, 0, :].transpose([0, 1, 3, 2])
                zi_v = zzp[:, :, :, 1, :].transpose([0, 1, 3, 2])
                for hc in range(2):
                    cols = slice(hc * 128, (hc + 1) * 128)
                    yp = ps4.tile([128, 2, W], f32, name="yps")
                    nc.tensor.matmul(yp, a4mt[:, cols], zr_v, start=True, stop=False)
                    nc.tensor.matmul(yp, b4mt[:, cols], zi_v, start=False, stop=True)
                    nc.gpsimd.tensor_copy(out=yt[:, :, hc, :], in_=yp)
                img0 = cl * nb + sg0 + 2 * pr
                nc.sync.dma_start(
                    out=y_d[img0:img0 + 2].rearrange(
                        "i (c p) w -> p i c w", c=2), in_=yt)

        # software pipeline: emit C of sg N after A+B of sg N+1 so the
        # in-order PE queue always has independent work ahead of the
        # S/D-dependent T3 matmuls.
        sgs = [(cl, sg0)
               for _rep in range(repeat)
               for cl in range(nc_loc)
               for sg0 in range(0, nb, g)]
        pend = None
        for cl, sg0 in sgs:
            st = emit_AB(cl, sg0)
            if pend is not None:
                emit_C(pend)
            pend = st
        emit_C(pend)
    nc.compile()
    return nc


def _prep_weights(w_real, w_imag, core, nc_loc=NC_LOC):
    """-> [nc_loc, 3, 2, 128, KP] bf16 (kind: Wr, Wi, -Wi; kwc chunks)."""
    warr = np.zeros((nc_loc, 3, 2, 128, KP), ml_dtypes.bfloat16)
    for cl in range(nc_loc):
        eff = _w_eff(w_real[0, core * nc_loc + cl], w_imag[0, core * nc_loc + cl])
        effT = eff.T                        # [256(kw), 129(kh)]
        for k, arr in enumerate([effT.real, effT.imag, -effT.imag]):
            a = np.zeros((2, 128, KP), np.float32)
            a[:, :, 0:KHF] = arr.reshape(2, 128, KHF).astype(np.float32)
            warr[cl, k] = a.astype(ml_dtypes.bfloat16)
    return warr


def _prep_core_inputs(x, w_real, w_imag, core):
    cs = slice(core * NC_LOC, (core + 1) * NC_LOC)
    xc = np.ascontiguousarray(x[:, cs].transpose(1, 0, 2, 3)).reshape(
        B * NC_LOC, H, W).astype(ml_dtypes.bfloat16)
    warr = _prep_weights(w_real, w_imag, core)
    return {"x": xc, "w": warr}


_NC_CACHE = {}


def kernel(x, w_real, w_imag):
    from concourse.bass_utils import run_bass_kernel_spmd
    x = np.asarray(x); w_real = np.asarray(w_real); w_imag = np.asarray(w_imag)
    key = "full"
    if key not in _NC_CACHE:
        _NC_CACHE[key] = build_nc()
    nc = _NC_CACHE[key]
    in_maps = [_prep_core_inputs(x, w_real, w_imag, i) for i in range(N_CORES)]
    res = run_bass_kernel_spmd(nc, in_maps, core_ids=list(range(N_CORES)))
    outs = []
    for i in range(N_CORES):
        yc = np.asarray(res.results[i]["y"]).reshape(
            NC_LOC, B, H, W).transpose(1, 0, 2, 3)
        outs.append(yc)
    return np.concatenate(outs, axis=1).astype(np.float32)


# revision 11
# speedup vs baseline: 1.0469x; 1.0469x over previous
"""FFT conv block (rfft2 -> per-channel complex multiply -> irfft2) on 8 trn2
cores — v4.2: bf16 I/O, T1 radix-2 DIF (host S/D prep), permuted kh layout
[ny | kh0 | evens | odds], nyquist folded into the dead imag-kh0 slot,
batched T3 moving matrices.

Per (b,c) image [256, 256] (sharded over channels, 8 ch/core):
  host: S = x_lo + x_hi, D = x_lo - x_hi over the h halves (radix-2 DIF),
      shipped as bf16 [2, 128, W] per image.
  T1: rfft over h via 4 half-size matmuls per (img, w-parity): S covers even
      kh (incl kh0 + nyquist), D covers odd kh. Column layout (129 r | 127 i):
      r: [ny, kh0, kh2..126, kh1..127], i: same minus ny/kh0 (zero cols),
      offset so the i->r column map is a uniform +2 shift.
  T2: radix-2 fft over w: lo = A + TB fully accumulated in PSUM (4 mm),
      TB separately (2 mm); ACT evicts lo -> bf16, DVE stt: hi = lo - 2*TB.
  wm: Yw = Y2 * Weff (bf16; 5 ops DVE, 1 Pool; weights broadcast over images)
  S/D + nyquist fold on Pool: col1(kh0) := col1 -/+ col0(ny) pairs the dead
      imag kh0 slot with the nyquist row (B4 row0 := a4n).
  T3: Z[kh, (r|i), m] via batched moving mats [Ce|Se], [-Se|Ce] etc (4 mm/img)
  T4: y = A4^T Zr + B4^T Zi (f32r, rows in perm order), evicted bf16, DMA out.
Verified vs reference: ~5.3e-3 rel rms (bf16-dominated), tolerance 2e-2.
"""
import sys
sys.path.insert(0, "/opt/trn_rl_repo")
import numpy as np
import ml_dtypes

B, C, H, W = 16, 64, 256, 256
KHF = H // 2 + 1          # 129
KP = 130                  # kh cols padded (129 valid + 1 pad)
N_CORES = 8
NC_LOC = C // N_CORES     # 8 channels per core
NB = B                    # 16 batch images per channel
G = 8                     # supergroup size (images of same channel)

# kh order in the packed column layout: col c holds kh PERM_FULL[c]
PERM_FULL = [128] + list(range(0, 128, 2)) + list(range(1, 128, 2))
PERM_MAIN = PERM_FULL[1:]          # T3 main cols 1..128 <-> T4 rows


def _consts_v42():
    f32 = np.float32
    bf = ml_dtypes.bfloat16
    h = np.arange(128)[:, None]
    khS = np.array([128] + list(range(0, 128, 2)))    # 65
    khSi = np.arange(2, 128, 2)                       # 63
    khD = np.arange(1, 128, 2)                        # 64
    M1S = np.concatenate([np.cos(2 * np.pi * h * khS / H),
                          -np.sin(2 * np.pi * h * khSi / H)], axis=1)  # [128,128]
    M1D = np.concatenate([np.cos(2 * np.pi * h * khD / H),
                          -np.sin(2 * np.pi * h * khD / H)], axis=1)   # [128,128]

    kw = np.arange(128)[None, :]
    we = np.arange(128)[:, None]
    cosE = np.cos(2 * np.pi * (2 * we) * kw / W)      # [we, kw]
    sinE = np.sin(2 * np.pi * (2 * we) * kw / W)
    cosO = np.cos(2 * np.pi * (2 * we + 1) * kw / W)
    sinO = np.sin(2 * np.pi * (2 * we + 1) * kw / W)

    kw2 = np.arange(128)[:, None]
    m = np.arange(128)[None, :]
    Ce = np.cos(2 * np.pi * kw2 * (2 * m) / W) / W
    Se = np.sin(2 * np.pi * kw2 * (2 * m) / W) / W
    Co = np.cos(2 * np.pi * kw2 * (2 * m + 1) / W) / W
    So = np.sin(2 * np.pi * kw2 * (2 * m + 1) / W) / W
    # batched T3 moving mats [kw, 2, m]: one matmul yields [zr | zi] halves
    Me1 = np.stack([Ce, Se], axis=1)
    Me2 = np.stack([-Se, Ce], axis=1)
    Mo1 = np.stack([Co, So], axis=1)
    Mo2 = np.stack([-So, Co], axis=1)

    kh2 = np.arange(KHF)[:, None]
    hp = np.arange(H)[None, :]
    t4 = 2 * np.pi * kh2 * hp / H
    alpha = np.where((kh2 == 0) | (kh2 == H // 2), 1.0, 2.0)
    A4 = alpha * np.cos(t4) / H                      # [129, 256]
    B4 = -alpha * np.sin(t4) / H
    A4m = A4[PERM_MAIN].astype(f32)                  # rows follow main col order
    B4m = B4[PERM_MAIN].astype(f32)
    B4m[0, :] = A4[128, :]    # nyquist row rides the dead imag kh0 slot
    return dict(
        M1S=M1S.astype(f32).astype(bf), M1D=M1D.astype(f32).astype(bf),
        cosE=cosE.astype(f32).astype(bf), sinE=sinE.astype(f32).astype(bf),
        nsinE=(-sinE).astype(f32).astype(bf),
        cosO=cosO.astype(f32).astype(bf), sinO=sinO.astype(f32).astype(bf),
        nsinO=(-sinO).astype(f32).astype(bf),
        Me1=Me1.astype(f32).astype(bf), Me2=Me2.astype(f32).astype(bf),
        Mo1=Mo1.astype(f32).astype(bf), Mo2=Mo2.astype(f32).astype(bf),
        A4m=A4m, B4m=B4m,
    )


def _w_eff(wr, wi):
    """wr, wi: [256(kh), 129(kw)] reference layout -> W_eff [129(kh), 256(kw)]."""
    w = wr.astype(np.float64) + 1j * wi.astype(np.float64)
    kh = np.arange(KHF)
    khc = (H - kh) % H
    eff = np.empty((KHF, W), dtype=np.complex128)
    eff[:, 0:W // 2 + 1] = w[0:KHF, :]
    for kwv in range(W // 2 + 1, W):
        eff[:, kwv] = np.conj(w[khc, W - kwv])
    for col in (0, W // 2):
        eff[:, col] = 0.5 * (w[kh, col] + np.conj(w[khc, col]))
    return eff                                        # [129(kh), 256(kw)] complex


def build_nc(nc_loc=NC_LOC, nb=NB, g=G, repeat=1):
    import concourse.mybir as mybir
    import concourse.tile as tile
    from concourse import bacc
    from contextlib import ExitStack

    f32, f32r = mybir.dt.float32, mybir.dt.float32r
    bf16 = mybir.dt.bfloat16
    n_img = nb * nc_loc
    npairs = g // 2
    CN = _consts_v42()

    nc = bacc.Bacc("TRN2", target_bir_lowering=False)
    x_d = nc.dram_tensor("x", [n_img, 2, 128, W], bf16, kind="ExternalInput")
    w_d = nc.dram_tensor("w", [nc_loc, 3, 2, 128, KP], bf16, kind="ExternalInput")
    y_d = nc.dram_tensor("y", [n_img, H, W], bf16, kind="ExternalOutput")

    dconst = {k: nc.inline_tensor(v, f"c_{k}") for k, v in CN.items()}

    with tile.TileContext(nc) as tc, ExitStack() as es:
        cpool = es.enter_context(tc.tile_pool(name="const", bufs=1))
        wpool = es.enter_context(tc.tile_pool(name="wpool", bufs=1))
        xrp = es.enter_context(tc.tile_pool(name="xr", bufs=2))
        y1p = es.enter_context(tc.tile_pool(name="y1", bufs=3))
        y2p = es.enter_context(tc.tile_pool(name="y2", bufs=2))
        ywp = es.enter_context(tc.tile_pool(name="yw", bufs=2))
        sdp = es.enter_context(tc.tile_pool(name="sd", bufs=2))
        zsb = es.enter_context(tc.tile_pool(name="zsb", bufs=3))
        ysb = es.enter_context(tc.tile_pool(name="ysb", bufs=3))
        ps1 = es.enter_context(tc.tile_pool(name="ps1", bufs=2, space="PSUM"))
        ps2 = es.enter_context(tc.tile_pool(name="ps2", bufs=1, space="PSUM"))
        ps3 = es.enter_context(tc.tile_pool(name="ps3", bufs=2, space="PSUM"))
        ps4 = es.enter_context(tc.tile_pool(name="ps4", bufs=2, space="PSUM"))

        def load_const(name, dt_):
            src = dconst[name]
            arr = CN[name]
            rows = arr.shape[0]
            inner = list(arr.shape[1:])
            src_is_bf = arr.dtype == ml_dtypes.bfloat16
            tiles = []
            nch = (rows + 127) // 128
            for ch in range(nch):
                r0, r1 = ch * 128, min((ch + 1) * 128, rows)
                t = cpool.tile([r1 - r0] + inner, dt_, name=f"c_{name}{ch}")
                out_ap = t[:] if src_is_bf else t[:].bitcast(f32)
                nc.sync.dma_start(out=out_ap, in_=src[r0:r1])
                tiles.append(t)
            return tiles if nch > 1 else tiles[0]

        m1St = load_const("M1S", bf16)             # [128, 128]
        m1Dt = load_const("M1D", bf16)
        cosEt = load_const("cosE", bf16)           # [128, 128] each
        sinEt = load_const("sinE", bf16)
        nsinEt = load_const("nsinE", bf16)
        cosOt = load_const("cosO", bf16)
        sinOt = load_const("sinO", bf16)
        nsinOt = load_const("nsinO", bf16)
        Me1t = load_const("Me1", bf16)             # [128, 2, 128] each
        Me2t = load_const("Me2", bf16)
        Mo1t = load_const("Mo1", bf16)
        Mo2t = load_const("Mo2", bf16)
        a4mt = load_const("A4m", f32r)             # [128, 256]
        b4mt = load_const("B4m", f32r)

        # weights: per channel [128(kw), 3(kind), 2(kwc), KP] bf16 on ACT queue
        wt = []
        for cl in range(nc_loc):
            t = wpool.tile([128, 3, 2, KP], bf16, name=f"w{cl}")
            nc.scalar.dma_start(
                out=t, in_=w_d[cl].rearrange("k c p f -> p k c f"))
            wt.append(t)

        def emit_AB(cl, sg0):
            """Phase A (per pair T1 DIF + T2) + B (wmul, S/D, ny-fold)."""
            y2r = y2p.tile([128, 2, g, KP], bf16, name="y2r")
            y2i = y2p.tile([128, 2, g, KP], bf16, name="y2i")
            for pr in range(npairs):
                img0 = cl * nb + sg0 + 2 * pr
                xr = xrp.tile([128, 2, 2, W], bf16, name="xr")
                nc.sync.dma_start(
                    out=xr,
                    in_=x_d[img0:img0 + 2].rearrange("i s p w -> p i s w"))
                # T1 DIF: 4 half-size matmuls per (img, parity); single-matmul
                # groups write disjoint column ranges of one PSUM bank.
                y1 = y1p.tile([128, 2, 2, 256], bf16, name="y1")
                for j in range(2):
                    t1 = ps1.tile([128, 2, 256], f32, name="t1ps")
                    for par in range(2):
                        sS = xr[:, j, 0, par::2]
                        sD = xr[:, j, 1, par::2]
                        nc.tensor.matmul(t1[:, par, 0:65], sS, m1St[:, 0:65],
                                         start=True, stop=True,
                                         skip_group_check=True)
                        nc.tensor.matmul(t1[:, par, 129:192], sS, m1St[:, 65:128],
                                         start=True, stop=True,
                                         skip_group_check=True)
                        nc.tensor.matmul(t1[:, par, 65:129], sD, m1Dt[:, 0:64],
                                         start=True, stop=True,
                                         skip_group_check=True)
                        nc.tensor.matmul(t1[:, par, 192:256], sD, m1Dt[:, 64:128],
                                         start=True, stop=True,
                                         skip_group_check=True)
                    nc.scalar.copy(out=y1[:, j], in_=t1)
                # T2 radix-2 over w: lo = A + TB accumulated in PSUM (4 mm,
                # i-cols land at a uniform +2 column shift), TB separately
                # (2 mm); ACT copy lo, DVE stt hi = lo - 2*TB.
                sl = slice(2 * pr, 2 * pr + 2)
                for half, y2t in ((0, y2r), (1, y2i)):
                    c1 = cosEt if half == 0 else nsinEt   # even-w, r cols
                    c2 = sinEt if half == 0 else cosEt    # even-w, i cols
                    o1 = cosOt if half == 0 else nsinOt   # odd-w, r cols
                    o2 = sinOt if half == 0 else cosOt    # odd-w, i cols
                    lo = ps2.tile([128, 2, KP], f32, name="lops")
                    tb = ps2.tile([128, 2, KP], f32, name="tbps")
                    nc.tensor.matmul(tb[:, :, 0:129], o1, y1[:, :, 1, 0:129],
                                     start=True, stop=False)
                    nc.tensor.matmul(tb[:, :, 2:129], o2, y1[:, :, 1, 129:256],
                                     start=False, stop=True)
                    nc.tensor.matmul(lo[:, :, 0:129], c1, y1[:, :, 0, 0:129],
                                     start=True, stop=False)
                    nc.tensor.matmul(lo[:, :, 2:129], c2, y1[:, :, 0, 129:256],
                                     start=False, stop=False)
                    nc.tensor.matmul(lo[:, :, 0:129], o1, y1[:, :, 1, 0:129],
                                     start=False, stop=False)
                    nc.tensor.matmul(lo[:, :, 2:129], o2, y1[:, :, 1, 129:256],
                                     start=False, stop=True)
                    nc.scalar.copy(out=y2t[:, 0, sl, 0:129], in_=lo[:, :, 0:129])
                    from concourse import mybir as _mb
                    nc.vector.scalar_tensor_tensor(
                        out=y2t[:, 1, sl, 0:129], in0=tb[:, :, 0:129],
                        scalar=-2.0, in1=y2t[:, 0, sl, 0:129],
                        op0=_mb.AluOpType.mult, op1=_mb.AluOpType.add)
            # ---- phase B: wmul + S/D + nyquist fold ----
            kv = slice(0, 129)
            wr_b = wt[cl][:, 0, :, :].unsqueeze(2).broadcast_to([128, 2, g, KP])
            wi_b = wt[cl][:, 1, :, :].unsqueeze(2).broadcast_to([128, 2, g, KP])
            nwi_b = wt[cl][:, 2, :, :].unsqueeze(2).broadcast_to([128, 2, g, KP])
            ta = ywp.tile([128, 2, g, KP], bf16, name="ta")
            tb = ywp.tile([128, 2, g, KP], bf16, name="tb")
            ta2 = ywp.tile([128, 2, g, KP], bf16, name="ta2")
            tb2 = ywp.tile([128, 2, g, KP], bf16, name="tb2")
            ywr = ywp.tile([128, 2, g, KP], bf16, name="ywr")
            ywi = ywp.tile([128, 2, g, KP], bf16, name="ywi")
            nc.vector.tensor_mul(ta[:, :, :, kv], y2r[:, :, :, kv], wr_b[:, :, :, kv])
            nc.vector.tensor_mul(tb[:, :, :, kv], y2i[:, :, :, kv], nwi_b[:, :, :, kv])
            nc.vector.tensor_add(ywr[:, :, :, kv], ta[:, :, :, kv], tb[:, :, :, kv])
            nc.vector.tensor_mul(ta2[:, :, :, kv], y2r[:, :, :, kv], wi_b[:, :, :, kv])
            nc.gpsimd.tensor_mul(tb2[:, :, :, kv], y2i[:, :, :, kv], wr_b[:, :, :, kv])
            nc.vector.tensor_add(ywi[:, :, :, kv], ta2[:, :, :, kv], tb2[:, :, :, kv])
            sr = sdp.tile([128, g, KP], bf16, name="sr")
            si = sdp.tile([128, g, KP], bf16, name="si")
            dr = sdp.tile([128, g, KP], bf16, name="dr")
            di = sdp.tile([128, g, KP], bf16, name="di")
            nc.gpsimd.tensor_add(sr[:, :, kv], ywr[:, 0, :, kv], ywr[:, 1, :, kv])
            nc.gpsimd.tensor_add(si[:, :, kv], ywi[:, 0, :, kv], ywi[:, 1, :, kv])
            nc.gpsimd.tensor_sub(dr[:, :, kv], ywr[:, 0, :, kv], ywr[:, 1, :, kv])
            nc.gpsimd.tensor_sub(di[:, :, kv], ywi[:, 0, :, kv], ywi[:, 1, :, kv])
            # nyquist fold: col1 (kh0) := col1 -/+ i*col0 (ny); writes col 1,
            # reads col 0 — order-independent.
            nc.gpsimd.tensor_sub(sr[:, :, 1], sr[:, :, 1], si[:, :, 0])
            nc.gpsimd.tensor_add(si[:, :, 1], si[:, :, 1], sr[:, :, 0])
            nc.gpsimd.tensor_sub(dr[:, :, 1], dr[:, :, 1], di[:, :, 0])
            nc.gpsimd.tensor_add(di[:, :, 1], di[:, :, 1], dr[:, :, 0])
            return dict(sr=sr, si=si, dr=dr, di=di, cl=cl, sg0=sg0)

        def emit_C(st):
            """Phase C: per image T3 (batched moving); per pair T4 + store."""
            sr, si, dr, di = st["sr"], st["si"], st["dr"], st["di"]
            cl, sg0 = st["cl"], st["sg0"]
            kmain = slice(1, 129)
            for pr in range(npairs):
                zzp = zsb.tile([128, 2, 2, 2, 128], f32r, name="zzp")
                for j in range(2):
                    gg = 2 * pr + j
                    z2 = ps3.tile([128, 2, 2, 128], f32, name="z2ps")
                    nc.tensor.matmul(z2[:, 0], sr[:, gg, kmain], Me1t, start=True, stop=False)
                    nc.tensor.matmul(z2[:, 0], si[:, gg, kmain], Me2t, start=False, stop=True)
                    nc.tensor.matmul(z2[:, 1], dr[:, gg, kmain], Mo1t, start=True, stop=False)
                    nc.tensor.matmul(z2[:, 1], di[:, gg, kmain], Mo2t, start=False, stop=True)
                    if j == 0:
                        nc.scalar.copy(out=zzp[:, j], in_=z2)
                    elif pr % 2 == 0:
                        nc.vector.tensor_copy(out=zzp[:, j], in_=z2)
                    else:
                        nc.scalar.copy(out=zzp[:, j], in_=z2)
                # T4: pair-batched; moving reads Z interleaved (m,eo swap)
                yt = ysb.tile([128, 2, 2, W], bf16, name="yt")
                zr_v = zzp[:, :, :, 0, :].transpose([0, 1, 3, 2])
                zi_v = zzp[:, :, :, 1, :].transpose([0, 1, 3, 2])
                for hc in range(2):
                    cols = slice(hc * 128, (hc + 1) * 128)
                    yp = ps4.tile([128, 2, W], f32, name="yps")
                    nc.tensor.matmul(yp, a4mt[:, cols], zr_v, start=True, stop=False)
                    nc.tensor.matmul(yp, b4mt[:, cols], zi_v, start=False, stop=True)
                    if hc == 0:
                        nc.scalar.copy(out=yt[:, :, hc, :], in_=yp)
                    else:
                        nc.vector.tensor_copy(out=yt[:, :, hc, :], in_=yp)
                img0 = cl * nb + sg0 + 2 * pr
                nc.sync.dma_start(
                    out=y_d[img0:img0 + 2].rearrange(
                        "i (c p) w -> p i c w", c=2), in_=yt)

        # software pipeline: emit C of sg N after A+B of sg N+1 so the
        # in-order PE queue always has independent work ahead of the
        # S/D-dependent T3 matmuls.
        sgs = [(cl, sg0)
               for _rep in range(repeat)
               for cl in range(nc_loc)
               for sg0 in range(0, nb, g)]
        pend = None
        for cl, sg0 in sgs:
            st = emit_AB(cl, sg0)
            if pend is not None:
                emit_C(pend)
            pend = st
        emit_C(pend)
    nc.compile()
    return nc


def _prep_weights(w_real, w_imag, core, nc_loc=NC_LOC):
    """-> [nc_loc, 3, 2, 128, KP] bf16 (kind: Wr, Wi, -Wi; kh cols in PERM
    order; kwc chunks)."""
    warr = np.zeros((nc_loc, 3, 2, 128, KP), ml_dtypes.bfloat16)
    perm = np.array(PERM_FULL)
    for cl in range(nc_loc):
        eff = _w_eff(w_real[0, core * nc_loc + cl], w_imag[0, core * nc_loc + cl])
        effT = eff[perm].T                  # [256(kw), 129(kh perm order)]
        for k, arr in enumerate([effT.real, effT.imag, -effT.imag]):
            a = np.zeros((2, 128, KP), np.float32)
            a[:, :, 0:KHF] = arr.reshape(2, 128, KHF).astype(np.float32)
            warr[cl, k] = a.astype(ml_dtypes.bfloat16)
    return warr


def _prep_core_inputs(x, w_real, w_imag, core):
    cs = slice(core * NC_LOC, (core + 1) * NC_LOC)
    xc = np.ascontiguousarray(x[:, cs].transpose(1, 0, 2, 3)).reshape(
        B * NC_LOC, H, W)
    xl, xh = xc[:, 0:128, :], xc[:, 128:256, :]
    xsd = np.stack([xl + xh, xl - xh], axis=1)        # [n_img, 2, 128, W]
    warr = _prep_weights(w_real, w_imag, core)
    return {"x": xsd.astype(ml_dtypes.bfloat16), "w": warr}


_NC_CACHE = {}


def kernel(x, w_real, w_imag):
    from concourse.bass_utils import run_bass_kernel_spmd
    x = np.asarray(x); w_real = np.asarray(w_real); w_imag = np.asarray(w_imag)
    key = "full"
    if key not in _NC_CACHE:
        _NC_CACHE[key] = build_nc()
    nc = _NC_CACHE[key]
    in_maps = [_prep_core_inputs(x, w_real, w_imag, i) for i in range(N_CORES)]
    res = run_bass_kernel_spmd(nc, in_maps, core_ids=list(range(N_CORES)))
    outs = []
    for i in range(N_CORES):
        yc = np.asarray(res.results[i]["y"]).reshape(
            NC_LOC, B, H, W).transpose(1, 0, 2, 3)
        outs.append(yc)
    return np.concatenate(outs, axis=1).astype(np.float32)


# revision 13
# speedup vs baseline: 1.5199x; 1.4518x over previous
"""FFT conv block (rfft2 -> per-channel complex multiply -> irfft2) on 8 trn2
cores — v4.2: bf16 I/O, T1 radix-2 DIF (host S/D prep), permuted kh layout
[ny | kh0 | evens | odds], nyquist folded into the dead imag-kh0 slot,
batched T3 moving matrices.

Per (b,c) image [256, 256] (sharded over channels, 8 ch/core):
  host: S = x_lo + x_hi, D = x_lo - x_hi over the h halves (radix-2 DIF),
      shipped as bf16 [2, 128, W] per image.
  T1: rfft over h via 4 half-size matmuls per (img, w-parity): S covers even
      kh (incl kh0 + nyquist), D covers odd kh. Column layout (129 r | 127 i):
      r: [ny, kh0, kh2..126, kh1..127], i: same minus ny/kh0 (zero cols),
      offset so the i->r column map is a uniform +2 shift.
  T2: radix-2 fft over w: lo = A + TB fully accumulated in PSUM (4 mm),
      TB separately (2 mm); ACT evicts lo -> bf16, DVE stt: hi = lo - 2*TB.
  wm: Yw = Y2 * Weff (bf16; 5 ops DVE, 1 Pool; weights broadcast over images)
  S/D + nyquist fold on Pool: col1(kh0) := col1 -/+ col0(ny) pairs the dead
      imag kh0 slot with the nyquist row (B4 row0 := a4n).
  T3: Z[kh, (r|i), m] via batched moving mats [Ce|Se], [-Se|Ce] etc (4 mm/img)
  T4: y = A4^T Zr + B4^T Zi (f32r, rows in perm order), evicted bf16, DMA out.
Verified vs reference: ~5.3e-3 rel rms (bf16-dominated), tolerance 2e-2.
"""
import sys
sys.path.insert(0, "/opt/trn_rl_repo")
import numpy as np
import ml_dtypes

B, C, H, W = 16, 64, 256, 256
KHF = H // 2 + 1          # 129
KP = 130                  # kh cols padded (129 valid + 1 pad)
N_CORES = 8
NC_LOC = C // N_CORES     # 8 channels per core
NB = B                    # 16 batch images per channel
G = 8                     # supergroup size (images of same channel)

# kh order in the packed column layout: col c holds kh PERM_FULL[c]
PERM_FULL = [128] + list(range(0, 128, 2)) + list(range(1, 128, 2))
PERM_MAIN = PERM_FULL[1:]          # T3 main cols 1..128 <-> T4 rows


def _consts_v42():
    f32 = np.float32
    bf = ml_dtypes.bfloat16
    h = np.arange(128)[:, None]
    khS = np.array([128] + list(range(0, 128, 2)))    # 65
    khSi = np.arange(2, 128, 2)                       # 63
    khD = np.arange(1, 128, 2)                        # 64
    M1S = np.concatenate([np.cos(2 * np.pi * h * khS / H),
                          -np.sin(2 * np.pi * h * khSi / H)], axis=1)  # [128,128]
    M1D = np.concatenate([np.cos(2 * np.pi * h * khD / H),
                          -np.sin(2 * np.pi * h * khD / H)], axis=1)   # [128,128]

    kw = np.arange(128)[None, :]
    we = np.arange(128)[:, None]
    cosE = np.cos(2 * np.pi * (2 * we) * kw / W)      # [we, kw]
    sinE = np.sin(2 * np.pi * (2 * we) * kw / W)
    cosO = np.cos(2 * np.pi * (2 * we + 1) * kw / W)
    sinO = np.sin(2 * np.pi * (2 * we + 1) * kw / W)

    kw2 = np.arange(128)[:, None]
    m = np.arange(128)[None, :]
    Ce = np.cos(2 * np.pi * kw2 * (2 * m) / W) / W
    Se = np.sin(2 * np.pi * kw2 * (2 * m) / W) / W
    Co = np.cos(2 * np.pi * kw2 * (2 * m + 1) / W) / W
    So = np.sin(2 * np.pi * kw2 * (2 * m + 1) / W) / W
    # batched T3 moving mats [kw, 2, m]: one matmul yields [zr | zi] halves
    Me1 = np.stack([Ce, Se], axis=1)
    Me2 = np.stack([-Se, Ce], axis=1)
    Mo1 = np.stack([Co, So], axis=1)
    Mo2 = np.stack([-So, Co], axis=1)

    kh2 = np.arange(KHF)[:, None]
    hp = np.arange(H)[None, :]
    t4 = 2 * np.pi * kh2 * hp / H
    alpha = np.where((kh2 == 0) | (kh2 == H // 2), 1.0, 2.0)
    A4 = alpha * np.cos(t4) / H                      # [129, 256]
    B4 = -alpha * np.sin(t4) / H
    A4m = A4[PERM_MAIN].astype(f32)                  # rows follow main col order
    B4m = B4[PERM_MAIN].astype(f32)
    B4m[0, :] = A4[128, :]    # nyquist row rides the dead imag kh0 slot
    A4m = A4m.astype(bf)
    B4m = B4m.astype(bf)
    return dict(
        M1S=M1S.astype(f32).astype(bf), M1D=M1D.astype(f32).astype(bf),
        cosE=cosE.astype(f32).astype(bf), sinE=sinE.astype(f32).astype(bf),
        nsinE=(-sinE).astype(f32).astype(bf),
        cosO=cosO.astype(f32).astype(bf), sinO=sinO.astype(f32).astype(bf),
        nsinO=(-sinO).astype(f32).astype(bf),
        Me1=Me1.astype(f32).astype(bf), Me2=Me2.astype(f32).astype(bf),
        Mo1=Mo1.astype(f32).astype(bf), Mo2=Mo2.astype(f32).astype(bf),
        A4m=A4m, B4m=B4m,
    )


def _w_eff(wr, wi):
    """wr, wi: [256(kh), 129(kw)] reference layout -> W_eff [129(kh), 256(kw)]."""
    w = wr.astype(np.float64) + 1j * wi.astype(np.float64)
    kh = np.arange(KHF)
    khc = (H - kh) % H
    eff = np.empty((KHF, W), dtype=np.complex128)
    eff[:, 0:W // 2 + 1] = w[0:KHF, :]
    for kwv in range(W // 2 + 1, W):
        eff[:, kwv] = np.conj(w[khc, W - kwv])
    for col in (0, W // 2):
        eff[:, col] = 0.5 * (w[kh, col] + np.conj(w[khc, col]))
    return eff                                        # [129(kh), 256(kw)] complex


def build_nc(nc_loc=NC_LOC, nb=NB, g=G, repeat=1):
    import concourse.mybir as mybir
    import concourse.tile as tile
    from concourse import bacc
    from contextlib import ExitStack

    f32, f32r = mybir.dt.float32, mybir.dt.float32r
    bf16 = mybir.dt.bfloat16
    n_img = nb * nc_loc
    npairs = g // 2
    CN = _consts_v42()

    nc = bacc.Bacc("TRN2", target_bir_lowering=False)
    x_d = nc.dram_tensor("x", [n_img, 2, 128, W], bf16, kind="ExternalInput")
    w_d = nc.dram_tensor("w", [nc_loc, 3, 2, 128, KP], bf16, kind="ExternalInput")
    y_d = nc.dram_tensor("y", [n_img, H, W], bf16, kind="ExternalOutput")

    dconst = {k: nc.inline_tensor(v, f"c_{k}") for k, v in CN.items()}

    with tile.TileContext(nc) as tc, ExitStack() as es:
        cpool = es.enter_context(tc.tile_pool(name="const", bufs=1))
        wpool = es.enter_context(tc.tile_pool(name="wpool", bufs=1))
        xrp = es.enter_context(tc.tile_pool(name="xr", bufs=2))
        y1p = es.enter_context(tc.tile_pool(name="y1", bufs=3))
        y2p = es.enter_context(tc.tile_pool(name="y2", bufs=2))
        ywp = es.enter_context(tc.tile_pool(name="yw", bufs=2))
        sdp = es.enter_context(tc.tile_pool(name="sd", bufs=2))
        zsb = es.enter_context(tc.tile_pool(name="zsb", bufs=3))
        ysb = es.enter_context(tc.tile_pool(name="ysb", bufs=3))
        ps1 = es.enter_context(tc.tile_pool(name="ps1", bufs=2, space="PSUM"))
        ps2 = es.enter_context(tc.tile_pool(name="ps2", bufs=1, space="PSUM"))
        ps3 = es.enter_context(tc.tile_pool(name="ps3", bufs=2, space="PSUM"))
        ps4 = es.enter_context(tc.tile_pool(name="ps4", bufs=2, space="PSUM"))

        def load_const(name, dt_):
            src = dconst[name]
            arr = CN[name]
            rows = arr.shape[0]
            inner = list(arr.shape[1:])
            src_is_bf = arr.dtype == ml_dtypes.bfloat16
            tiles = []
            nch = (rows + 127) // 128
            for ch in range(nch):
                r0, r1 = ch * 128, min((ch + 1) * 128, rows)
                t = cpool.tile([r1 - r0] + inner, dt_, name=f"c_{name}{ch}")
                out_ap = t[:] if src_is_bf else t[:].bitcast(f32)
                nc.sync.dma_start(out=out_ap, in_=src[r0:r1])
                tiles.append(t)
            return tiles if nch > 1 else tiles[0]

        m1St = load_const("M1S", bf16)             # [128, 128]
        m1Dt = load_const("M1D", bf16)
        cosEt = load_const("cosE", bf16)           # [128, 128] each
        sinEt = load_const("sinE", bf16)
        nsinEt = load_const("nsinE", bf16)
        cosOt = load_const("cosO", bf16)
        sinOt = load_const("sinO", bf16)
        nsinOt = load_const("nsinO", bf16)
        Me1t = load_const("Me1", bf16)             # [128, 2, 128] each
        Me2t = load_const("Me2", bf16)
        Mo1t = load_const("Mo1", bf16)
        Mo2t = load_const("Mo2", bf16)
        a4mt = load_const("A4m", bf16)             # [128, 256]
        b4mt = load_const("B4m", bf16)

        # weights: per channel [128(kw), 3(kind), 2(kwc), KP] bf16 on ACT queue
        wt = []
        for cl in range(nc_loc):
            t = wpool.tile([128, 3, 2, KP], bf16, name=f"w{cl}")
            nc.scalar.dma_start(
                out=t, in_=w_d[cl].rearrange("k c p f -> p k c f"))
            wt.append(t)

        def emit_AB(cl, sg0):
            """Phase A (per pair T1 DIF + T2) + B (wmul, S/D, ny-fold)."""
            y2r = y2p.tile([128, 2, g, KP], bf16, name="y2r")
            y2i = y2p.tile([128, 2, g, KP], bf16, name="y2i")
            for pr in range(npairs):
                img0 = cl * nb + sg0 + 2 * pr
                xr = xrp.tile([128, 2, 2, W], bf16, name="xr")
                nc.sync.dma_start(
                    out=xr,
                    in_=x_d[img0:img0 + 2].rearrange("i s p w -> p i s w"))
                # T1 DIF: 4 half-size matmuls per (img, parity); single-matmul
                # groups write disjoint column ranges of one PSUM bank.
                y1 = y1p.tile([128, 2, 2, 256], bf16, name="y1")
                for j in range(2):
                    t1 = ps1.tile([128, 2, 256], f32, name="t1ps")
                    for par in range(2):
                        sS = xr[:, j, 0, par::2]
                        sD = xr[:, j, 1, par::2]
                        nc.tensor.matmul(t1[:, par, 0:65], sS, m1St[:, 0:65],
                                         start=True, stop=True,
                                         skip_group_check=True)
                        nc.tensor.matmul(t1[:, par, 129:192], sS, m1St[:, 65:128],
                                         start=True, stop=True,
                                         skip_group_check=True)
                        nc.tensor.matmul(t1[:, par, 65:129], sD, m1Dt[:, 0:64],
                                         start=True, stop=True,
                                         skip_group_check=True)
                        nc.tensor.matmul(t1[:, par, 192:256], sD, m1Dt[:, 64:128],
                                         start=True, stop=True,
                                         skip_group_check=True)
                    nc.scalar.copy(out=y1[:, j], in_=t1)
                # T2 radix-2 over w: lo = A + TB accumulated in PSUM (4 mm,
                # i-cols land at a uniform +2 column shift), TB separately
                # (2 mm); ACT copy lo, DVE stt hi = lo - 2*TB.
                sl = slice(2 * pr, 2 * pr + 2)
                for half, y2t in ((0, y2r), (1, y2i)):
                    c1 = cosEt if half == 0 else nsinEt   # even-w, r cols
                    c2 = sinEt if half == 0 else cosEt    # even-w, i cols
                    o1 = cosOt if half == 0 else nsinOt   # odd-w, r cols
                    o2 = sinOt if half == 0 else cosOt    # odd-w, i cols
                    lo = ps2.tile([128, 2, KP], f32, name="lops")
                    tb = ps2.tile([128, 2, KP], f32, name="tbps")
                    # ordered so each stationary (o1, o2, c1, c2) loads once
                    nc.tensor.matmul(tb[:, :, 0:129], o1, y1[:, :, 1, 0:129],
                                     start=True, stop=False)
                    nc.tensor.matmul(lo[:, :, 0:129], o1, y1[:, :, 1, 0:129],
                                     start=True, stop=False)
                    nc.tensor.matmul(tb[:, :, 2:129], o2, y1[:, :, 1, 129:256],
                                     start=False, stop=True)
                    nc.tensor.matmul(lo[:, :, 2:129], o2, y1[:, :, 1, 129:256],
                                     start=False, stop=False)
                    nc.tensor.matmul(lo[:, :, 0:129], c1, y1[:, :, 0, 0:129],
                                     start=False, stop=False)
                    nc.tensor.matmul(lo[:, :, 2:129], c2, y1[:, :, 0, 129:256],
                                     start=False, stop=True)
                    nc.scalar.copy(out=y2t[:, 0, sl, 0:129], in_=lo[:, :, 0:129])
                    from concourse import mybir as _mb
                    nc.vector.scalar_tensor_tensor(
                        out=y2t[:, 1, sl, 0:129], in0=tb[:, :, 0:129],
                        scalar=-2.0, in1=y2t[:, 0, sl, 0:129],
                        op0=_mb.AluOpType.mult, op1=_mb.AluOpType.add)
            # ---- phase B: wmul + S/D + nyquist fold ----
            kv = slice(0, 129)
            wr_b = wt[cl][:, 0, :, :].unsqueeze(2).broadcast_to([128, 2, g, KP])
            wi_b = wt[cl][:, 1, :, :].unsqueeze(2).broadcast_to([128, 2, g, KP])
            nwi_b = wt[cl][:, 2, :, :].unsqueeze(2).broadcast_to([128, 2, g, KP])
            ta = ywp.tile([128, 2, g, KP], bf16, name="ta")
            tb = ywp.tile([128, 2, g, KP], bf16, name="tb")
            ta2 = ywp.tile([128, 2, g, KP], bf16, name="ta2")
            tb2 = ywp.tile([128, 2, g, KP], bf16, name="tb2")
            ywr = ywp.tile([128, 2, g, KP], bf16, name="ywr")
            ywi = ywp.tile([128, 2, g, KP], bf16, name="ywi")
            nc.vector.tensor_mul(ta[:, :, :, kv], y2r[:, :, :, kv], wr_b[:, :, :, kv])
            nc.vector.tensor_mul(tb[:, :, :, kv], y2i[:, :, :, kv], nwi_b[:, :, :, kv])
            nc.vector.tensor_add(ywr[:, :, :, kv], ta[:, :, :, kv], tb[:, :, :, kv])
            nc.vector.tensor_mul(ta2[:, :, :, kv], y2r[:, :, :, kv], wi_b[:, :, :, kv])
            nc.gpsimd.tensor_mul(tb2[:, :, :, kv], y2i[:, :, :, kv], wr_b[:, :, :, kv])
            nc.vector.tensor_add(ywi[:, :, :, kv], ta2[:, :, :, kv], tb2[:, :, :, kv])
            sr = sdp.tile([128, g, KP], bf16, name="sr")
            si = sdp.tile([128, g, KP], bf16, name="si")
            dr = sdp.tile([128, g, KP], bf16, name="dr")
            di = sdp.tile([128, g, KP], bf16, name="di")
            nc.gpsimd.tensor_add(sr[:, :, kv], ywr[:, 0, :, kv], ywr[:, 1, :, kv])
            nc.gpsimd.tensor_add(si[:, :, kv], ywi[:, 0, :, kv], ywi[:, 1, :, kv])
            nc.gpsimd.tensor_sub(dr[:, :, kv], ywr[:, 0, :, kv], ywr[:, 1, :, kv])
            nc.gpsimd.tensor_sub(di[:, :, kv], ywi[:, 0, :, kv], ywi[:, 1, :, kv])
            # nyquist fold: col1 (kh0) := col1 -/+ i*col0 (ny); writes col 1,
            # reads col 0 — order-independent.
            nc.gpsimd.tensor_sub(sr[:, :, 1], sr[:, :, 1], si[:, :, 0])
            nc.gpsimd.tensor_add(si[:, :, 1], si[:, :, 1], sr[:, :, 0])
            nc.gpsimd.tensor_sub(dr[:, :, 1], dr[:, :, 1], di[:, :, 0])
            nc.gpsimd.tensor_add(di[:, :, 1], di[:, :, 1], dr[:, :, 0])
            return dict(sr=sr, si=si, dr=dr, di=di, cl=cl, sg0=sg0)

        def emit_C(st):
            """Phase C: per image T3 (batched moving); per pair T4 + store."""
            sr, si, dr, di = st["sr"], st["si"], st["dr"], st["di"]
            cl, sg0 = st["cl"], st["sg0"]
            kmain = slice(1, 129)
            for pr in range(npairs):
                zzp = zsb.tile([128, 2, 2, 2, 128], bf16, name="zzp")
                for j in range(2):
                    gg = 2 * pr + j
                    z2 = ps3.tile([128, 2, 2, 128], f32, name="z2ps")
                    nc.tensor.matmul(z2[:, 0], sr[:, gg, kmain], Me1t, start=True, stop=False)
                    nc.tensor.matmul(z2[:, 0], si[:, gg, kmain], Me2t, start=False, stop=True)
                    nc.tensor.matmul(z2[:, 1], dr[:, gg, kmain], Mo1t, start=True, stop=False)
                    nc.tensor.matmul(z2[:, 1], di[:, gg, kmain], Mo2t, start=False, stop=True)
                    if j == 0:
                        nc.scalar.copy(out=zzp[:, j], in_=z2)
                    elif pr % 2 == 0:
                        nc.vector.tensor_copy(out=zzp[:, j], in_=z2)
                    else:
                        nc.scalar.copy(out=zzp[:, j], in_=z2)
                # T4: pair-batched; moving reads Z interleaved (m,eo swap)
                yt = ysb.tile([128, 2, 2, W], bf16, name="yt")
                zr_v = zzp[:, :, :, 0, :].transpose([0, 1, 3, 2])
                zi_v = zzp[:, :, :, 1, :].transpose([0, 1, 3, 2])
                for hc in range(2):
                    cols = slice(hc * 128, (hc + 1) * 128)
                    yp = ps4.tile([128, 2, W], f32, name="yps")
                    nc.tensor.matmul(yp, a4mt[:, cols], zr_v, start=True, stop=False)
                    nc.tensor.matmul(yp, b4mt[:, cols], zi_v, start=False, stop=True)
                    if hc == 0:
                        nc.scalar.copy(out=yt[:, :, hc, :], in_=yp)
                    else:
                        nc.vector.tensor_copy(out=yt[:, :, hc, :], in_=yp)
                img0 = cl * nb + sg0 + 2 * pr
                nc.sync.dma_start(
                    out=y_d[img0:img0 + 2].rearrange(
                        "i (c p) w -> p i c w", c=2), in_=yt)

        # software pipeline: emit C of sg N after A+B of sg N+1 so the
        # in-order PE queue always has independent work ahead of the
        # S/D-dependent T3 matmuls.
        sgs = [(cl, sg0)
               for _rep in range(repeat)
               for cl in range(nc_loc)
               for sg0 in range(0, nb, g)]
        pend = None
        for cl, sg0 in sgs:
            st = emit_AB(cl, sg0)
            if pend is not None:
                emit_C(pend)
            pend = st
        emit_C(pend)
    nc.compile()
    return nc


def _prep_weights(w_real, w_imag, core, nc_loc=NC_LOC):
    """-> [nc_loc, 3, 2, 128, KP] bf16 (kind: Wr, Wi, -Wi; kh cols in PERM
    order; kwc chunks)."""
    warr = np.zeros((nc_loc, 3, 2, 128, KP), ml_dtypes.bfloat16)
    perm = np.array(PERM_FULL)
    for cl in range(nc_loc):
        eff = _w_eff(w_real[0, core * nc_loc + cl], w_imag[0, core * nc_loc + cl])
        effT = eff[perm].T                  # [256(kw), 129(kh perm order)]
        for k, arr in enumerate([effT.real, effT.imag, -effT.imag]):
            a = np.zeros((2, 128, KP), np.float32)
            a[:, :, 0:KHF] = arr.reshape(2, 128, KHF).astype(np.float32)
            warr[cl, k] = a.astype(ml_dtypes.bfloat16)
    return warr


def _prep_core_inputs(x, w_real, w_imag, core):
    cs = slice(core * NC_LOC, (core + 1) * NC_LOC)
    xc = np.ascontiguousarray(x[:, cs].transpose(1, 0, 2, 3)).reshape(
        B * NC_LOC, H, W)
    xl, xh = xc[:, 0:128, :], xc[:, 128:256, :]
    xsd = np.stack([xl + xh, xl - xh], axis=1)        # [n_img, 2, 128, W]
    warr = _prep_weights(w_real, w_imag, core)
    return {"x": xsd.astype(ml_dtypes.bfloat16), "w": warr}


_NC_CACHE = {}


def kernel(x, w_real, w_imag):
    from concourse.bass_utils import run_bass_kernel_spmd
    x = np.asarray(x); w_real = np.asarray(w_real); w_imag = np.asarray(w_imag)
    key = "full"
    if key not in _NC_CACHE:
        _NC_CACHE[key] = build_nc()
    nc = _NC_CACHE[key]
    in_maps = [_prep_core_inputs(x, w_real, w_imag, i) for i in range(N_CORES)]
    res = run_bass_kernel_spmd(nc, in_maps, core_ids=list(range(N_CORES)))
    outs = []
    for i in range(N_CORES):
        yc = np.asarray(res.results[i]["y"]).reshape(
            NC_LOC, B, H, W).transpose(1, 0, 2, 3)
        outs.append(yc)
    return np.concatenate(outs, axis=1).astype(np.float32)
